# revision 9
# baseline (speedup 1.0000x reference)
"""Bahdanau additive attention on Trainium2, SPMD over 8 NeuronCores.

Per batch element b:
    q_proj = query @ Ws.T            (T, H)
    e_proj = enc   @ Wh.T            (S, H)
    scores[t, s] = sum_h v[h] * tanh(q_proj[t, h] + e_proj[s, h])
    attn = masked softmax over s     (mask: s < src_lengths[b])
    out[t, h] = sum_s attn[t, s] * enc[s, h]

Sharding: every core runs all B=8 batches over TLOC = T/8 = 16 of the
t rows (t-sharded, load-balanced); per-batch source extents from
src_lengths are baked into the compiled program (compiled lazily per
lengths tuple). No collectives; host divides by the softmax
denominator and reassembles.

Pipeline (per core): PE computes both projections (e_proj stays in
PSUM); the DVE forms the (T,S,H) tanh argument with broadcast
(stride-0) tensor_tensor adds, one instruction per (batch, t-block,
j-pair); ACT runs tanh on [128, 2*8*len] blocks and the per-batch exp;
PE contracts tanh against v with FWL-padded [128,128] stationaries
(garbage rows are masked after exp) and computes denominator + context.
Engines are strict FIFO, so stages are emitted lag-scheduled
(exp at lag 2, den/ctx at lag 3, copies/DMA at lag 4).
"""

from contextlib import ExitStack

import numpy as np

import concourse.bass as bass
import concourse.bacc as bacc
import concourse.mybir as mybir
import concourse.tile as tile
from concourse.bass_utils import run_bass_kernel_spmd

B, T, S, H = 8, 128, 256, 512
NCORES = 8
P = 128
KT = H // P      # 4 feature tiles
ST = S // P      # 2 source tiles
TLOC = T // NCORES   # 16 t rows per core
TBV = 8              # t-block for ACT batching

dt = mybir.dt
AF = mybir.ActivationFunctionType


def _plan(lengths):
    lengths = [int(x) for x in lengths]
    plan = []
    col = 0
    for b, ln in enumerate(lengths):
        ln_c = min((ln + 1) // 2 * 2, S)   # even-pad compute extent
        halves = []                        # (u, m_bu, col_offset)
        for u in range(ST):
            m = min(P, ln - u * P)
            if m > 0:
                halves.append((u, m, col))
                col += TLOC
        plan.append({"b": b, "len": ln, "len_c": ln_c, "halves": halves})
    return plan, col


def _build_kernel_v5(tc, ctx, aps, plan, ncols, stat_fp8=False, gp_every=0):
    nc = tc.nc
    f32 = dt.float32
    f16 = dt.float16
    sdt = dt.float8e4 if stat_fp8 else f16

    const = ctx.enter_context(tc.tile_pool(name="const", bufs=1))
    psQ = ctx.enter_context(tc.tile_pool(name="psQ", bufs=1, space="PSUM"))
    psE = ctx.enter_context(tc.tile_pool(name="psE", bufs=4, space="PSUM"))
    psS = ctx.enter_context(tc.tile_pool(name="psS", bufs=1, space="PSUM"))
    psC = ctx.enter_context(tc.tile_pool(name="psC", bufs=2, space="PSUM"))
    tanh_pool = ctx.enter_context(tc.tile_pool(name="tanh", bufs=4))

    # Batch processing order: 2nd-shortest first (fast pipeline fill),
    # then descending length, shortest last (smallest drain tail).
    ln_sorted = sorted(range(B), key=lambda b: plan[b]["len_c"])
    order = [ln_sorted[1]] + ln_sorted[2:][::-1] + [ln_sorted[0]]

    # ---- input DMAs (SP queue, in priority order) ----------------------
    wsT_sb = const.tile([P, KT, H], f16)
    whT_sb = const.tile([P, KT, H], f16)
    vcol_sb = const.tile([P, KT], f16)
    qTs_sb = const.tile([P, B, KT, TLOC], f16)
    encT_all = const.tile([P, B, KT, S], f16)
    enc_all = const.tile([P, B, ST, H], f16)
    # Critical-path inputs on the sync queue; the rest fan out over the
    # gpsimd queue so the streams run in parallel.
    encT_r = aps["encTs"].rearrange("b p x -> b p x")
    encs_r = aps["encs"].rearrange("b p x -> b p x")
    whT_r = aps["WhT"].rearrange("p (k o) -> p k o", o=H)
    wsT_r = aps["WsT"].rearrange("p (k o) -> p k o", o=H)
    nc.sync.dma_start(whT_sb[:, :, 0:H // 2], whT_r[:, :, 0:H // 2])
    nc.sync.dma_start(wsT_sb[:, :, 0:H // 2], wsT_r[:, :, 0:H // 2])
    nc.sync.dma_start(qTs_sb[:].rearrange("p b k t -> p (b k t)"), aps["queryTs"][:, :])
    nc.sync.dma_start(encT_all[:, order[0]].rearrange("p k s -> p (k s)"),
                      encT_r[order[0]])
    nc.sync.dma_start(whT_sb[:, :, H // 2:H], whT_r[:, :, H // 2:H])
    nc.sync.dma_start(wsT_sb[:, :, H // 2:H], wsT_r[:, :, H // 2:H])
    nc.sync.dma_start(vcol_sb[:], aps["vcol"][:, :])
    maskT_sb = const.tile([P, B, ST], f32)
    nc.sync.dma_start(maskT_sb[:].rearrange("p b u -> p (b u)"), aps["maskT"][:, :])
    for b in order[1:3]:
        nc.sync.dma_start(encT_all[:, b].rearrange("p k s -> p (k s)"), encT_r[b])
    # encT for later batches and enc_all (only needed by the context
    # matmuls) are DMA'd inside the batch loop so the startup wave stays
    # small and the critical weights land early.

    ones_sb = const.tile([P, 1], f16)
    nc.vector.memset(ones_sb[:], 1.0)

    # Fixed tout buffer (rotating slots) zeroed per slot on GPSIMD; the
    # first two slots before its DMA work so the first tanh isn't gated
    # by one huge memset.
    NBUF = 3
    tout_all = const.tile([P, NBUF, KT, TBV, S], sdt)
    for i in range(NBUF):
        nc.gpsimd.memset(tout_all[:, i].bitcast(dt.uint32), 0)

    # ---- e projection: per batch, result STAYS in PSUM -----------------
    # Two [P, 2, 256] tiles per batch (one bank each) so no matmul output
    # crosses a PSUM bank. The broadcast adds read these directly.
    e_projT = {}

    def emit_eproj(b):
        ln_c = plan[b]["len_c"]
        tiles = []
        for jp in range(2):
            ep_ps = psE.tile([P, 2, 1, 256], f32, tag="ep", name=f"ep{b}_{jp}")
            for jh in range(2):
                j = jp * 2 + jh
                for k in range(KT):
                    nc.tensor.matmul(
                        ep_ps[:, jh, 0, 0:ln_c],
                        lhsT=whT_sb[:, k, j * P:(j + 1) * P],
                        rhs=encT_all[:, b, k, 0:ln_c],
                        start=(k == 0), stop=(k == KT - 1))
            tiles.append(ep_ps)
        e_projT[b] = tiles

    # ---- persistent softmax / output tiles -----------------------------
    # expT is fp32 so garbage rows (padded-stationary scores up to ~|v|_1)
    # cannot overflow at exp; the mask-multiply zeroes them while still
    # finite, then a cheap cast produces the fp16 operand for den/ctx.
    scT_ps = psS.tile([P, ncols], f32, name="scT")
    # Zero once so rows >= m (never written by the exact-width score
    # matmuls) hold 0, not uninitialized PSUM; exp(0)=1 is then masked.
    nc.vector.memset(scT_ps[:], 0.0)
    expT32_sb = const.tile([P, ncols], f32)
    expT_sb = const.tile([P, ncols], f16)
    ctxT_sb = const.tile([P, B, KT * TLOC + ST * TLOC], f32)
    ones2_sb = const.tile([P, P], f16)
    nc.vector.memset(ones2_sb[:], 1.0)

    # ---- per-batch stages ----------------------------------------------
    slot_ctr = [0]

    def emit_scores(pb):
        b, ln_c = pb["b"], pb["len_c"]
        for tb in range(TLOC // TBV):
            t0 = tb * TBV
            slot = slot_ctr[0] % NBUF
            slot_ctr[0] += 1
            tin = tanh_pool.tile([P, KT, TBV, ln_c], f16, tag="tin",
                                 name=f"tin{b}_{tb}")
            tout = tout_all[:, slot]
            for jp in range(2):
                # one broadcast add per j-pair: [P, (j:2), (tl:8), (s:ln)]
                ep_b = e_projT[b][jp][:, :, :, 0:ln_c]        # [P,2,1,ln]
                qp_b = q_projT[:, 2 * jp:2 * jp + 2,
                               b * TLOC + t0:b * TLOC + t0 + TBV, :]  # [P,2,8,1]
                ab, bb = bass.broadcast_tensor_aps(ep_b, qp_b)
                nc.vector.tensor_add(tin[:, 2 * jp:2 * jp + 2], ab, bb)
                # tanh per j-pair so ACT starts after half the adds
                nc.scalar.activation(tout[:, 2 * jp:2 * jp + 2, :, 0:ln_c],
                                     tin[:, 2 * jp:2 * jp + 2], AF.Tanh)
            for tl in range(TBV):
                for (u, m, col) in pb["halves"]:
                    cc = col + t0 + tl
                    for j in range(KT):
                        # exact-width stationary: LDWEIGHTS cost scales
                        # with columns, so don't pad partial halves
                        nc.tensor.matmul(
                            scT_ps[0:m, cc:cc + 1],
                            lhsT=tout[:, j, tl, u * P:u * P + m],
                            rhs=vcol_sb[:, j:j + 1],
                            start=(j == 0), stop=(j == KT - 1))

    def emit_exp(pb):
        c0 = pb["halves"][0][2]
        nb = TLOC * len(pb["halves"])
        nc.scalar.activation(expT32_sb[:, c0:c0 + nb], scT_ps[:, c0:c0 + nb], AF.Exp)

    tail_state = {}

    def emit_tail_mm(pb):
        b = pb["b"]
        c0 = pb["halves"][0][2]
        nb = TLOC * len(pb["halves"])
        for (u, m, col) in pb["halves"]:
            if m < P:
                nc.vector.tensor_scalar_mul(
                    expT32_sb[:, col:col + TLOC], expT32_sb[:, col:col + TLOC],
                    maskT_sb[:, b, u:u + 1])
        nc.vector.tensor_copy(expT_sb[:, c0:c0 + nb], expT32_sb[:, c0:c0 + nb])
        # den lives in the tail columns of the ctx PSUM tile; an all-ones
        # [128,128] stationary broadcasts the column-sum to every
        # partition, so ctx+den leave in ONE copy and ONE DMA per batch.
        ctx_ps = psC.tile([P, KT * TLOC + ST * TLOC], f32, tag="ctx", name=f"ctx{b}")
        nc.tensor.matmul(ctx_ps[:, KT * TLOC:KT * TLOC + nb],
                         lhsT=ones2_sb[:], rhs=expT_sb[:, c0:c0 + nb])
        nh = len(pb["halves"])
        for hb in range(KT):
            for i, (u, m, col) in enumerate(pb["halves"]):
                nc.tensor.matmul(
                    ctx_ps[:, hb * TLOC:(hb + 1) * TLOC],
                    lhsT=enc_all[:, b, u, hb * P:(hb + 1) * P],
                    rhs=expT_sb[:, col:col + TLOC],
                    start=(i == 0), stop=(i == nh - 1))
        tail_state[b] = (ctx_ps, c0, nb)

    OUTW = KT * TLOC + ST * TLOC

    def emit_tail_out(pb):
        b = pb["b"]
        ctx_ps, c0, nb = tail_state.pop(b)
        w = KT * TLOC + nb
        nc.scalar.copy(ctxT_sb[:, b, 0:w], ctx_ps[:, 0:w])
        nc.sync.dma_start(
            aps["outb"][:, b * OUTW:b * OUTW + w], ctxT_sb[:, b, 0:w])

    # ---- emission schedule ---------------------------------------------
    emit_eproj(order[0])
    # q projection: all batches at once, weights shared per (j,k); one
    # PSUM tile for all j (regions are disjoint, groups sequential) and a
    # single copy out. Trailing singleton dim so slices broadcast
    # against [P,2,1,ln] APs.
    q_projT = const.tile([P, KT, B * TLOC, 1], f32)
    qp_ps = psQ.tile([P, KT, B * TLOC], f32, tag="qp", name="qp")
    for jp in range(2):
        for j in (2 * jp, 2 * jp + 1):
            for k in range(KT):
                nc.tensor.matmul(
                    qp_ps[:, j, :], lhsT=wsT_sb[:, k, j * P:(j + 1) * P],
                    rhs=qTs_sb[:, :, k, :], start=(k == 0), stop=(k == KT - 1))
        nc.scalar.copy(
            q_projT[:, 2 * jp:2 * jp + 2].rearrange("p k t o -> p (k t o)"),
            qp_ps[:, 2 * jp:2 * jp + 2].rearrange("p k t -> p (k t)"))
    emit_eproj(order[1])
    for s in range(B + 4):
        if s + 3 < B:
            # encT for the batch entering eproj two steps from now
            b_pre = order[s + 3]
            nc.sync.dma_start(
                encT_all[:, b_pre].rearrange("p k s -> p (k s)"), encT_r[b_pre])
        if s < B:
            # enc_all only feeds the context matmuls (3 steps later)
            b_now = order[s]
            nc.sync.dma_start(
                enc_all[:, b_now].rearrange("p u h -> p (u h)"), encs_r[b_now])
        if s + 2 < B:
            emit_eproj(order[s + 2])
        if s < B:
            emit_scores(plan[order[s]])
        if 2 <= s < B + 2:
            emit_exp(plan[order[s - 2]])
        if 3 <= s < B + 3:
            emit_tail_mm(plan[order[s - 3]])
        if 4 <= s:
            emit_tail_out(plan[order[s - 4]])


def build_nc_v5(lengths, stat_fp8=False, gp_every=0):
    plan, ncols = _plan(lengths)
    nc = bacc.Bacc("TRN2", target_bir_lowering=False, debug=False)
    aps = {
        "queryTs": nc.dram_tensor("queryTs", [P, B * KT * TLOC], dt.float16, kind="ExternalInput").ap(),
        "encTs": nc.dram_tensor("encTs", [B, P, KT * S], dt.float16, kind="ExternalInput").ap(),
        "encs": nc.dram_tensor("encs", [B, P, ST * H], dt.float16, kind="ExternalInput").ap(),
        "WsT": nc.dram_tensor("WsT", [P, KT * H], dt.float16, kind="ExternalInput").ap(),
        "WhT": nc.dram_tensor("WhT", [P, KT * H], dt.float16, kind="ExternalInput").ap(),
        "vcol": nc.dram_tensor("vcol", [P, KT], dt.float16, kind="ExternalInput").ap(),
        "maskT": nc.dram_tensor("maskT", [P, B * ST], dt.float32, kind="ExternalInput").ap(),
        "outb": nc.dram_tensor(
            "outb", [P, B * (KT * TLOC + ST * TLOC)], dt.float32,
            kind="ExternalOutput").ap(),
    }
    with ExitStack() as ctx:
        with tile.TileContext(nc) as tc:
            _build_kernel_v5(tc, ctx, aps, plan, ncols, stat_fp8, gp_every)
            ctx.close()
    nc.compile()
    return nc, plan, ncols


def make_in_maps_v5(query, encoder_outputs, src_lengths, Ws, Wh, v):
    wsT = np.ascontiguousarray(Ws.T).astype(np.float16)
    whT = np.ascontiguousarray(Wh.T).astype(np.float16)
    vcol = np.ascontiguousarray(np.asarray(v, np.float32).reshape(KT, P).T).astype(np.float16)
    enc16 = np.asarray(encoder_outputs, np.float16)
    # encTs[b, p, (k, s)] = enc[b, s, k*128+p]
    encTs = np.ascontiguousarray(
        enc16.transpose(0, 2, 1).reshape(B, KT, P, S).transpose(0, 2, 1, 3)
        .reshape(B, P, KT * S))
    # encs[b, p, (u, h)] = enc[b, u*128+p, h]
    encs = np.ascontiguousarray(
        enc16.reshape(B, ST, P, H).transpose(0, 2, 1, 3).reshape(B, P, ST * H))
    wsT = np.ascontiguousarray(
        wsT.reshape(KT, P, H).transpose(1, 0, 2).reshape(P, KT * H))
    whT = np.ascontiguousarray(
        whT.reshape(KT, P, H).transpose(1, 0, 2).reshape(P, KT * H))
    qT = np.asarray(query, np.float16).transpose(0, 2, 1)  # [B, H, T]
    # maskT[p, (b, u)] = 1.0 if u*128+p < len[b] else 0.0
    maskT = np.zeros((P, B, ST), np.float32)
    for b in range(B):
        m01 = (np.arange(S) < int(src_lengths[b])).astype(np.float32)
        maskT[:, b, :] = m01.reshape(ST, P).T
    maskT = np.ascontiguousarray(maskT.reshape(P, B * ST))
    in_maps = []
    for c in range(NCORES):
        # queryTs[p, (b, k, t)] = query[b, c*16+t, k*128+p]
        qc = qT[:, :, c * TLOC:(c + 1) * TLOC]
        qc = np.ascontiguousarray(
            qc.reshape(B, KT, P, TLOC).transpose(2, 0, 1, 3).reshape(P, B * KT * TLOC))
        in_maps.append({
            "queryTs": qc,
            "encTs": encTs, "encs": encs,
            "WsT": wsT, "WhT": whT, "vcol": vcol, "maskT": maskT,
        })
    return in_maps


def combine_v5(results, plan, ncols):
    out = np.zeros((B, T, H), np.float32)
    outw = KT * TLOC + ST * TLOC
    for c in range(NCORES):
        blob = results[c]["outb"].reshape(P, B, outw)
        for pb in plan:
            b = pb["b"]
            nh = len(pb["halves"])
            ctxT = blob[:, b, 0:KT * TLOC].reshape(P, KT, TLOC)  # [p, hb, t]
            den = blob[0, b, KT * TLOC:KT * TLOC + TLOC * nh]
            d = den.reshape(nh, TLOC).sum(axis=0)
            cpart = ctxT.transpose(1, 0, 2).reshape(H, TLOC)     # [h, t]
            out[b, c * TLOC:(c + 1) * TLOC, :] = (cpart / d[None, :]).T
    return out


_NC_V6 = {}


def _kernel_v6(query, encoder_outputs, src_lengths, Ws, Wh, v):
    key = tuple(int(x) for x in np.asarray(src_lengths))
    if key not in _NC_V6:
        _NC_V6[key] = build_nc_v5(key)
    nc, plan, ncols = _NC_V6[key]
    in_maps = make_in_maps_v5(query, encoder_outputs, src_lengths, Ws, Wh, v)
    res = run_bass_kernel_spmd(nc, in_maps, core_ids=list(range(NCORES)))
    return combine_v5(res.results, plan, ncols).astype(np.float32)


def kernel(query, encoder_outputs, src_lengths, Ws, Wh, v):
    try:
        return _kernel_v6(query, encoder_outputs, src_lengths, Ws, Wh, v)
    except Exception:
        return _kernel_v4(query, encoder_outputs, src_lengths, Ws, Wh, v)


# ===================== v4: length-aware T-sharded build =====================
# Every core runs the SAME program over ALL B batches but only TLOC=T/8 of
# the t rows; per-batch s-extents (from src_lengths) are baked in as static
# code, so dead source positions cost nothing. Masking falls out of exact
# stationary widths plus a -40 PSUM memset (exp -> 0). The context is
# produced transposed (ctxT[h, (hb, b, t)]) and UNNORMALIZED together with
# the per-(b,u,t) denominator row; the host does the final divide and
# re-layout. Compiled lazily per src_lengths tuple.

TLOC = T // NCORES   # 16 t rows per core
TBV = 8              # t-block for ACT batching (2 blocks per batch)


def _plan(lengths):
    lengths = [int(x) for x in lengths]
    plan = []
    col = 0
    for b, ln in enumerate(lengths):
        ln_c = (ln + 1) // 2 * 2          # pad compute extent to even (f16 align)
        halves = []                        # (u, m_bu, col_offset)
        for u in range(ST):
            m = min(P, ln - u * P)
            if m > 0:
                halves.append((u, m, col))
                col += TLOC
        plan.append({"b": b, "len": ln, "len_c": min(ln_c, S), "halves": halves})
    return plan, col                      # col = total scT columns (16 * sum halves)


def _build_kernel_v4(tc, ctx, aps, plan, ncols):
    nc = tc.nc
    f32 = dt.float32
    f16 = dt.float16

    const = ctx.enter_context(tc.tile_pool(name="const", bufs=1))
    psP = ctx.enter_context(tc.tile_pool(name="psP", bufs=2, space="PSUM"))
    psS = ctx.enter_context(tc.tile_pool(name="psS", bufs=1, space="PSUM"))

    # ---- inputs: all host-repacked partition-major, contiguous rows ----
    wsT_sb = const.tile([P, KT, H], f16)
    whT_sb = const.tile([P, KT, H], f16)
    vcol_sb = const.tile([P, KT], f16)
    nc.sync.dma_start(vcol_sb[:], aps["vcol"][:, :])
    qTs_sb = const.tile([P, B, KT, TLOC], f16)
    enc_all = const.tile([P, B, ST, H], f16)
    encT_all = const.tile([P, B, KT, S], f16)
    qTs_r = aps["queryTs"].rearrange("b p x -> b p x")
    encT_r = aps["encTs"].rearrange("b p x -> b p x")
    encs_r = aps["encs"].rearrange("b p x -> b p x")
    for b in range(B):
        nc.sync.dma_start(
            qTs_sb[:, b].rearrange("p k t -> p (k t)"), qTs_r[b])
        nc.sync.dma_start(
            encT_all[:, b].rearrange("p k s -> p (k s)"), encT_r[b])
        nc.sync.dma_start(
            enc_all[:, b].rearrange("p u h -> p (u h)"), encs_r[b])
        if b == 0:
            nc.sync.dma_start(
                wsT_sb[:].rearrange("p k o -> p (k o)"), aps["WsT"][:, :])
            nc.sync.dma_start(
                whT_sb[:].rearrange("p k o -> p (k o)"), aps["WhT"][:, :])
    encT_sb = [encT_all[:, b] for b in range(B)]
    u_index = {}
    for pb in plan:
        for i, (u, m, _) in enumerate(pb["halves"]):
            u_index[(pb["b"], i)] = u

    # ---- scores: scT[s, col(b,u,t)] in one PSUM tile --------------------
    scT_ps = psS.tile([P, ncols], f32, name="scT")
    nc.vector.memset(scT_ps[:], -40.0)
    tanh_pool = ctx.enter_context(tc.tile_pool(name="tanh", bufs=4))

    # Projections are software-pipelined ONE BATCH AHEAD of the score
    # loop: PE's in-order stream would otherwise place proj(b+1) after
    # scores(b) (which wait on tanh(b)), stalling the next batch's adds
    # and opening ACT gaps at every batch boundary.
    q_projT = const.tile([P, B, KT, TLOC], f32)
    e_projT = []

    def emit_proj(pb):
        b, ln_c = pb["b"], pb["len_c"]
        for j in range(KT):
            qp_ps = psP.tile([P, TLOC], f32, tag="qp", name=f"qp{b}_{j}")
            for k in range(KT):
                nc.tensor.matmul(
                    qp_ps[:], lhsT=wsT_sb[:, k, j * P:(j + 1) * P],
                    rhs=qTs_sb[:, b, k, :], start=(k == 0), stop=(k == KT - 1))
            nc.scalar.copy(q_projT[:, b, j, :], qp_ps[:])
        ep = const.tile([P, KT, ln_c], f16, name=f"eprojT{b}", tag=f"eprojT{b}")
        for j in range(KT):
            ep_ps = psP.tile([P, S], f32, tag="ep", name=f"ep{b}_{j}")
            for k in range(KT):
                nc.tensor.matmul(
                    ep_ps[:, 0:ln_c], lhsT=whT_sb[:, k, j * P:(j + 1) * P],
                    rhs=encT_sb[b][:, k, 0:ln_c], start=(k == 0), stop=(k == KT - 1))
            nc.scalar.copy(ep[:, j, :], ep_ps[:, 0:ln_c])
        e_projT.append(ep)

    emit_proj(plan[0])
    for bi, pb in enumerate(plan):
        b, ln_c = pb["b"], pb["len_c"]
        if bi + 1 < len(plan):
            emit_proj(plan[bi + 1])

        for tb in range(TLOC // TBV):
            t0 = tb * TBV
            tin = tanh_pool.tile([P, TBV, KT, ln_c], f16, tag="tin", name=f"tin{b}_{tb}")
            tout = tanh_pool.tile([P, TBV, KT, ln_c], f16, tag="tout", name=f"tout{b}_{tb}")
            for tl in range(TBV):
                for j in range(KT):
                    nc.vector.tensor_scalar_add(
                        tin[:, tl, j, :], e_projT[b][:, j, :],
                        q_projT[:, b, j, t0 + tl:t0 + tl + 1])
            nc.scalar.activation(tout[:], tin[:], AF.Tanh)
            for tl in range(TBV):
                for (u, m, col) in pb["halves"]:
                    cc = col + t0 + tl
                    for j in range(KT):
                        nc.tensor.matmul(
                            scT_ps[0:m, cc:cc + 1],
                            lhsT=tout[:, tl, j, u * P:u * P + m],
                            rhs=vcol_sb[:, j:j + 1],
                            start=(j == 0), stop=(j == KT - 1))

    # ---- exp + denominator ---------------------------------------------
    expT_sb = const.tile([P, ncols], f16)
    nc.scalar.activation(expT_sb[:], scT_ps[:], AF.Exp)
    ones_sb = const.tile([P, 1], f16)
    nc.vector.memset(ones_sb[:], 1.0)
    den_ps = psP.tile([1, ncols], f32, tag="den")
    nc.tensor.matmul(den_ps[:], lhsT=ones_sb[:], rhs=expT_sb[:])
    den_sb = const.tile([1, ncols], f32)
    nc.vector.tensor_copy(den_sb[:], den_ps[:])
    nc.sync.dma_start(aps["den"][:, :], den_sb[:])

    # ---- context (transposed, unnormalized) -----------------------------
    # ctxT_ps[p, (hb, b, t)] = sum_s enc[b][s, hb*128+p] * expT[s, col(b,u,t)]
    ctxT_ps = psS.tile([P, KT * B * TLOC], f32, name="ctxT")
    for pb in plan:
        b = pb["b"]
        nh = len(pb["halves"])
        for hb in range(KT):
            for i, (u, m, col) in enumerate(pb["halves"]):
                nc.tensor.matmul(
                    ctxT_ps[:, (hb * B + b) * TLOC:(hb * B + b + 1) * TLOC],
                    lhsT=enc_all[:, b, u, hb * P:(hb + 1) * P],
                    rhs=expT_sb[:, col:col + TLOC],
                    start=(i == 0), stop=(i == nh - 1))
    ctxT_sb = const.tile([P, KT * B * TLOC], f32)
    nc.vector.tensor_copy(ctxT_sb[:], ctxT_ps[:])
    nc.sync.dma_start(aps["ctxT"][:, :], ctxT_sb[:])


def build_nc_v4(lengths):
    plan, ncols = _plan(lengths)
    nc = bacc.Bacc("TRN2", target_bir_lowering=False, debug=False)
    aps = {
        "queryTs": nc.dram_tensor("queryTs", [B, P, KT * TLOC], dt.float16, kind="ExternalInput").ap(),
        "encTs": nc.dram_tensor("encTs", [B, P, KT * S], dt.float16, kind="ExternalInput").ap(),
        "encs": nc.dram_tensor("encs", [B, P, ST * H], dt.float16, kind="ExternalInput").ap(),
        "WsT": nc.dram_tensor("WsT", [P, KT * H], dt.float16, kind="ExternalInput").ap(),
        "WhT": nc.dram_tensor("WhT", [P, KT * H], dt.float16, kind="ExternalInput").ap(),
        "vcol": nc.dram_tensor("vcol", [P, KT], dt.float16, kind="ExternalInput").ap(),
        "den": nc.dram_tensor("den", [1, ncols], dt.float32, kind="ExternalOutput").ap(),
        "ctxT": nc.dram_tensor("ctxT", [P, KT * B * TLOC], dt.float32, kind="ExternalOutput").ap(),
    }
    with ExitStack() as ctx:
        with tile.TileContext(nc) as tc:
            _build_kernel_v4(tc, ctx, aps, plan, ncols)
            ctx.close()
    nc.compile()
    return nc, plan, ncols


def make_in_maps_v4(query, encoder_outputs, src_lengths, Ws, Wh, v):
    wsT = np.ascontiguousarray(Ws.T).astype(np.float16)
    whT = np.ascontiguousarray(Wh.T).astype(np.float16)
    vcol = np.ascontiguousarray(np.asarray(v, np.float32).reshape(KT, P).T).astype(np.float16)
    enc16 = np.asarray(encoder_outputs, np.float16)
    # encTs[b, p, (k, s)] = enc[b, s, k*128+p]
    encTs = np.ascontiguousarray(
        enc16.transpose(0, 2, 1).reshape(B, KT, P, S).transpose(0, 2, 1, 3)
        .reshape(B, P, KT * S))
    # encs[b, p, (u, h)] = enc[b, u*128+p, h]
    encs = np.ascontiguousarray(
        enc16.reshape(B, ST, P, H).transpose(0, 2, 1, 3).reshape(B, P, ST * H))
    # wsT2[p, (k, o)] = Ws.T[k*128+p, o]
    wsT = np.ascontiguousarray(
        wsT.reshape(KT, P, H).transpose(1, 0, 2).reshape(P, KT * H))
    whT = np.ascontiguousarray(
        whT.reshape(KT, P, H).transpose(1, 0, 2).reshape(P, KT * H))
    qT = np.asarray(query, np.float16).transpose(0, 2, 1)  # [B, H, T]
    in_maps = []
    for c in range(NCORES):
        # queryTs[b, p, (k, t)] = query[b, c*16+t, k*128+p]
        qc = qT[:, :, c * TLOC:(c + 1) * TLOC]
        qc = np.ascontiguousarray(
            qc.reshape(B, KT, P, TLOC).transpose(0, 2, 1, 3).reshape(B, P, KT * TLOC))
        in_maps.append({
            "queryTs": qc,
            "encTs": encTs, "encs": encs,
            "WsT": wsT, "WhT": whT, "vcol": vcol,
        })
    return in_maps


def combine_v4(results, plan, ncols):
    out = np.zeros((B, T, H), np.float32)
    for c in range(NCORES):
        ctxT = results[c]["ctxT"].reshape(P, KT, B, TLOC)   # [p, hb, b, t]
        den = results[c]["den"].reshape(ncols)
        for pb in plan:
            b = pb["b"]
            d = np.zeros(TLOC, np.float32)
            for (u, m, col) in pb["halves"]:
                d += den[col:col + TLOC]
            # ctx[t, h] with h = hb*128 + p
            cpart = ctxT[:, :, b, :].transpose(1, 0, 2).reshape(H, TLOC)
            out[b, c * TLOC:(c + 1) * TLOC, :] = (cpart / d[None, :]).T
    return out


_NC_V4 = {}


def _kernel_v4(query, encoder_outputs, src_lengths, Ws, Wh, v):
    key = tuple(int(x) for x in np.asarray(src_lengths))
    if key not in _NC_V4:
        _NC_V4[key] = build_nc_v4(key)
    nc, plan, ncols = _NC_V4[key]
    in_maps = make_in_maps_v4(query, encoder_outputs, src_lengths, Ws, Wh, v)
    res = run_bass_kernel_spmd(nc, in_maps, core_ids=list(range(NCORES)))
    return combine_v4(res.results, plan, ncols).astype(np.float32)




# revision 11
# speedup vs baseline: 1.0636x; 1.0636x over previous
"""Bahdanau additive attention on Trainium2, SPMD over 8 NeuronCores.

Per batch element b:
    q_proj = query @ Ws.T            (T, H)
    e_proj = enc   @ Wh.T            (S, H)
    scores[t, s] = sum_h v[h] * tanh(q_proj[t, h] + e_proj[s, h])
    attn = masked softmax over s     (mask: s < src_lengths[b])
    out[t, h] = sum_s attn[t, s] * enc[s, h]

Sharding: every core runs all B=8 batches over TLOC = T/8 = 16 of the
t rows (t-sharded, load-balanced); per-batch source extents from
src_lengths are baked into the compiled program (compiled lazily per
lengths tuple). No collectives; host divides by the softmax
denominator and reassembles.

Pipeline (per core): PE computes both projections (e_proj stays in
PSUM); the DVE forms the (T,S,H) tanh argument with broadcast
(stride-0) tensor_tensor adds, one instruction per (batch, t-block,
j-pair); ACT runs tanh on [128, 2*8*len] blocks and the per-batch exp;
PE contracts tanh against v with FWL-padded [128,128] stationaries
(garbage rows are masked after exp) and computes denominator + context.
Engines are strict FIFO, so stages are emitted lag-scheduled
(exp at lag 2, den/ctx at lag 3, copies/DMA at lag 4).
"""

from contextlib import ExitStack

import numpy as np

import concourse.bass as bass
import concourse.bacc as bacc
import concourse.mybir as mybir
import concourse.tile as tile
from concourse.bass_utils import run_bass_kernel_spmd

B, T, S, H = 8, 128, 256, 512
NCORES = 8
P = 128
KT = H // P      # 4 feature tiles
ST = S // P      # 2 source tiles
TLOC = T // NCORES   # 16 t rows per core
TBV = 8              # t-block for ACT batching

dt = mybir.dt
AF = mybir.ActivationFunctionType


def _plan(lengths):
    lengths = [int(x) for x in lengths]
    plan = []
    col = 0
    for b, ln in enumerate(lengths):
        ln_c = min((ln + 1) // 2 * 2, S)   # even-pad compute extent
        halves = []                        # (u, m_bu, col_offset)
        for u in range(ST):
            m = min(P, ln - u * P)
            if m > 0:
                halves.append((u, m, col))
                col += TLOC
        plan.append({"b": b, "len": ln, "len_c": ln_c, "halves": halves})
    return plan, col


def _build_kernel_v5(tc, ctx, aps, plan, ncols, stat_fp8=False, gp_every=0):
    nc = tc.nc
    f32 = dt.float32
    f16 = dt.float16
    sdt = dt.float8e4 if stat_fp8 else f16

    const = ctx.enter_context(tc.tile_pool(name="const", bufs=1))
    psQ = ctx.enter_context(tc.tile_pool(name="psQ", bufs=1, space="PSUM"))
    psE = ctx.enter_context(tc.tile_pool(name="psE", bufs=4, space="PSUM"))
    psS = ctx.enter_context(tc.tile_pool(name="psS", bufs=1, space="PSUM"))
    psC = ctx.enter_context(tc.tile_pool(name="psC", bufs=2, space="PSUM"))
    tanh_pool = ctx.enter_context(tc.tile_pool(name="tanh", bufs=4))

    # Batch processing order: 2nd-shortest first (fast pipeline fill),
    # then descending length, shortest last (smallest drain tail).
    ln_sorted = sorted(range(B), key=lambda b: plan[b]["len_c"])
    order = [ln_sorted[1]] + ln_sorted[2:][::-1] + [ln_sorted[0]]

    # ---- input DMAs (SP queue, in priority order) ----------------------
    wsT_sb = const.tile([P, KT, H], f16)
    whT_sb = const.tile([P, KT, H], f16)
    vcol_sb = const.tile([P, KT], f16)
    qTs_sb = const.tile([P, B, KT, TLOC], f16)
    encT_all = const.tile([P, B, KT, S], f16)
    enc_all = const.tile([P, B, ST, H], f16)
    # Critical-path inputs on the sync queue; the rest fan out over the
    # gpsimd queue so the streams run in parallel.
    encT_r = aps["encTs"].rearrange("b p x -> b p x")
    encs_r = aps["encs"].rearrange("b p x -> b p x")
    whT_r = aps["WhT"].rearrange("p (k o) -> p k o", o=H)
    wsT_r = aps["WsT"].rearrange("p (k o) -> p k o", o=H)
    nc.sync.dma_start(whT_sb[:, :, 0:H // 2], whT_r[:, :, 0:H // 2])
    nc.sync.dma_start(wsT_sb[:, :, 0:H // 2], wsT_r[:, :, 0:H // 2])
    nc.sync.dma_start(qTs_sb[:].rearrange("p b k t -> p (b k t)"), aps["queryTs"][:, :])
    nc.sync.dma_start(encT_all[:, order[0]].rearrange("p k s -> p (k s)"),
                      encT_r[order[0]])
    nc.sync.dma_start(whT_sb[:, :, H // 2:H], whT_r[:, :, H // 2:H])
    nc.sync.dma_start(wsT_sb[:, :, H // 2:H], wsT_r[:, :, H // 2:H])
    nc.sync.dma_start(vcol_sb[:], aps["vcol"][:, :])
    maskT_sb = const.tile([P, B, ST], f32)
    nc.sync.dma_start(maskT_sb[:].rearrange("p b u -> p (b u)"), aps["maskT"][:, :])
    for b in order[1:3]:
        nc.sync.dma_start(encT_all[:, b].rearrange("p k s -> p (k s)"), encT_r[b])
    # encT for later batches and enc_all (only needed by the context
    # matmuls) are DMA'd inside the batch loop so the startup wave stays
    # small and the critical weights land early.

    ones_sb = const.tile([P, 1], f16)
    nc.vector.memset(ones_sb[:], 1.0)

    # Fixed tout buffer (rotating slots) zeroed per slot on GPSIMD; the
    # first two slots before its DMA work so the first tanh isn't gated
    # by one huge memset.
    NBUF = 3
    tout_all = const.tile([P, NBUF, KT, TBV, S], sdt)
    for i in range(NBUF):
        nc.gpsimd.memset(tout_all[:, i].bitcast(dt.uint32), 0)

    # ---- e projection: per batch, result STAYS in PSUM -----------------
    # Two [P, 2, 256] tiles per batch (one bank each) so no matmul output
    # crosses a PSUM bank. The broadcast adds read these directly.
    e_projT = {}

    def emit_eproj(b):
        ln_c = plan[b]["len_c"]
        tiles = []
        for jp in range(2):
            ep_ps = psE.tile([P, 2, 1, 256], f32, tag="ep", name=f"ep{b}_{jp}")
            for jh in range(2):
                j = jp * 2 + jh
                for k in range(KT):
                    nc.tensor.matmul(
                        ep_ps[:, jh, 0, 0:ln_c],
                        lhsT=whT_sb[:, k, j * P:(j + 1) * P],
                        rhs=encT_all[:, b, k, 0:ln_c],
                        start=(k == 0), stop=(k == KT - 1))
            tiles.append(ep_ps)
        e_projT[b] = tiles

    # ---- persistent softmax / output tiles -----------------------------
    # expT is fp32 so garbage rows (padded-stationary scores up to ~|v|_1)
    # cannot overflow at exp; the mask-multiply zeroes them while still
    # finite, then a cheap cast produces the fp16 operand for den/ctx.
    scT_ps = psS.tile([P, ncols], f32, name="scT")
    expT32_sb = const.tile([P, ncols], f32)
    expT_sb = const.tile([P, ncols], f16)
    ctxT_sb = const.tile([P, B, KT * TLOC + ST * TLOC], f32)
    ones2_sb = const.tile([P, P], f16)
    nc.vector.memset(ones2_sb[:], 1.0)

    # ---- per-batch stages ----------------------------------------------
    slot_ctr = [0]

    def emit_scores(pb):
        b, ln_c = pb["b"], pb["len_c"]
        for tb in range(TLOC // TBV):
            t0 = tb * TBV
            slot = slot_ctr[0] % NBUF
            slot_ctr[0] += 1
            tin = tanh_pool.tile([P, KT, TBV, ln_c], f16, tag="tin",
                                 name=f"tin{b}_{tb}")
            tout = tout_all[:, slot]
            for jp in range(2):
                # one broadcast add per j-pair: [P, (j:2), (tl:8), (s:ln)]
                ep_b = e_projT[b][jp][:, :, :, 0:ln_c]        # [P,2,1,ln]
                qp_b = q_projT[:, 2 * jp:2 * jp + 2,
                               b * TLOC + t0:b * TLOC + t0 + TBV, :]  # [P,2,8,1]
                ab, bb = bass.broadcast_tensor_aps(ep_b, qp_b)
                nc.vector.tensor_add(tin[:, 2 * jp:2 * jp + 2], ab, bb)
                # tanh per j-pair so ACT starts after half the adds
                nc.scalar.activation(tout[:, 2 * jp:2 * jp + 2, :, 0:ln_c],
                                     tin[:, 2 * jp:2 * jp + 2], AF.Tanh)
            for tl in range(TBV):
                for (u, m, col) in pb["halves"]:
                    cc = col + t0 + tl
                    for j in range(KT):
                        nc.tensor.matmul(
                            scT_ps[:, cc:cc + 1],
                            lhsT=tout[:, j, tl, u * P:(u + 1) * P],
                            rhs=vcol_sb[:, j:j + 1],
                            start=(j == 0), stop=(j == KT - 1))

    def emit_exp(pb):
        c0 = pb["halves"][0][2]
        nb = TLOC * len(pb["halves"])
        nc.scalar.activation(expT32_sb[:, c0:c0 + nb], scT_ps[:, c0:c0 + nb], AF.Exp)

    tail_state = {}

    def emit_tail_mm(pb):
        b = pb["b"]
        c0 = pb["halves"][0][2]
        nb = TLOC * len(pb["halves"])
        for (u, m, col) in pb["halves"]:
            if m < P:
                nc.vector.tensor_scalar_mul(
                    expT32_sb[:, col:col + TLOC], expT32_sb[:, col:col + TLOC],
                    maskT_sb[:, b, u:u + 1])
        nc.vector.tensor_copy(expT_sb[:, c0:c0 + nb], expT32_sb[:, c0:c0 + nb])
        # den lives in the tail columns of the ctx PSUM tile; an all-ones
        # [128,128] stationary broadcasts the column-sum to every
        # partition, so ctx+den leave in ONE copy and ONE DMA per batch.
        ctx_ps = psC.tile([P, KT * TLOC + ST * TLOC], f32, tag="ctx", name=f"ctx{b}")
        nc.tensor.matmul(ctx_ps[:, KT * TLOC:KT * TLOC + nb],
                         lhsT=ones2_sb[:], rhs=expT_sb[:, c0:c0 + nb])
        nh = len(pb["halves"])
        for hb in range(KT):
            for i, (u, m, col) in enumerate(pb["halves"]):
                nc.tensor.matmul(
                    ctx_ps[:, hb * TLOC:(hb + 1) * TLOC],
                    lhsT=enc_all[:, b, u, hb * P:(hb + 1) * P],
                    rhs=expT_sb[:, col:col + TLOC],
                    start=(i == 0), stop=(i == nh - 1))
        tail_state[b] = (ctx_ps, c0, nb)

    OUTW = KT * TLOC + ST * TLOC

    def emit_tail_out(pb):
        b = pb["b"]
        ctx_ps, c0, nb = tail_state.pop(b)
        w = KT * TLOC + nb
        nc.scalar.copy(ctxT_sb[:, b, 0:w], ctx_ps[:, 0:w])
        nc.sync.dma_start(
            aps["outb"][:, b * OUTW:b * OUTW + w], ctxT_sb[:, b, 0:w])

    # ---- emission schedule ---------------------------------------------
    emit_eproj(order[0])
    # q projection: all batches at once, weights shared per (j,k); one
    # PSUM tile for all j (regions are disjoint, groups sequential) and a
    # single copy out. Trailing singleton dim so slices broadcast
    # against [P,2,1,ln] APs.
    q_projT = const.tile([P, KT, B * TLOC, 1], f32)
    qp_ps = psQ.tile([P, KT, B * TLOC], f32, tag="qp", name="qp")
    for jp in range(2):
        for j in (2 * jp, 2 * jp + 1):
            for k in range(KT):
                nc.tensor.matmul(
                    qp_ps[:, j, :], lhsT=wsT_sb[:, k, j * P:(j + 1) * P],
                    rhs=qTs_sb[:, :, k, :], start=(k == 0), stop=(k == KT - 1))
        nc.scalar.copy(
            q_projT[:, 2 * jp:2 * jp + 2].rearrange("p k t o -> p (k t o)"),
            qp_ps[:, 2 * jp:2 * jp + 2].rearrange("p k t -> p (k t)"))
    emit_eproj(order[1])
    for s in range(B + 4):
        if s + 3 < B:
            # encT for the batch entering eproj two steps from now
            b_pre = order[s + 3]
            nc.sync.dma_start(
                encT_all[:, b_pre].rearrange("p k s -> p (k s)"), encT_r[b_pre])
        if s < B:
            # enc_all only feeds the context matmuls (3 steps later)
            b_now = order[s]
            nc.sync.dma_start(
                enc_all[:, b_now].rearrange("p u h -> p (u h)"), encs_r[b_now])
        if s + 2 < B:
            emit_eproj(order[s + 2])
        if s < B:
            emit_scores(plan[order[s]])
        if 2 <= s < B + 2:
            emit_exp(plan[order[s - 2]])
        if 3 <= s < B + 3:
            emit_tail_mm(plan[order[s - 3]])
        if 4 <= s:
            emit_tail_out(plan[order[s - 4]])


def build_nc_v5(lengths, stat_fp8=False, gp_every=0):
    plan, ncols = _plan(lengths)
    nc = bacc.Bacc("TRN2", target_bir_lowering=False, debug=False)
    aps = {
        "queryTs": nc.dram_tensor("queryTs", [P, B * KT * TLOC], dt.float16, kind="ExternalInput").ap(),
        "encTs": nc.dram_tensor("encTs", [B, P, KT * S], dt.float16, kind="ExternalInput").ap(),
        "encs": nc.dram_tensor("encs", [B, P, ST * H], dt.float16, kind="ExternalInput").ap(),
        "WsT": nc.dram_tensor("WsT", [P, KT * H], dt.float16, kind="ExternalInput").ap(),
        "WhT": nc.dram_tensor("WhT", [P, KT * H], dt.float16, kind="ExternalInput").ap(),
        "vcol": nc.dram_tensor("vcol", [P, KT], dt.float16, kind="ExternalInput").ap(),
        "maskT": nc.dram_tensor("maskT", [P, B * ST], dt.float32, kind="ExternalInput").ap(),
        "outb": nc.dram_tensor(
            "outb", [P, B * (KT * TLOC + ST * TLOC)], dt.float32,
            kind="ExternalOutput").ap(),
    }
    with ExitStack() as ctx:
        with tile.TileContext(nc) as tc:
            _build_kernel_v5(tc, ctx, aps, plan, ncols, stat_fp8, gp_every)
            ctx.close()
    nc.compile()
    return nc, plan, ncols


def make_in_maps_v5(query, encoder_outputs, src_lengths, Ws, Wh, v):
    wsT = np.ascontiguousarray(Ws.T).astype(np.float16)
    whT = np.ascontiguousarray(Wh.T).astype(np.float16)
    vcol = np.ascontiguousarray(np.asarray(v, np.float32).reshape(KT, P).T).astype(np.float16)
    enc16 = np.asarray(encoder_outputs, np.float16)
    # encTs[b, p, (k, s)] = enc[b, s, k*128+p]
    encTs = np.ascontiguousarray(
        enc16.transpose(0, 2, 1).reshape(B, KT, P, S).transpose(0, 2, 1, 3)
        .reshape(B, P, KT * S))
    # encs[b, p, (u, h)] = enc[b, u*128+p, h]
    encs = np.ascontiguousarray(
        enc16.reshape(B, ST, P, H).transpose(0, 2, 1, 3).reshape(B, P, ST * H))
    wsT = np.ascontiguousarray(
        wsT.reshape(KT, P, H).transpose(1, 0, 2).reshape(P, KT * H))
    whT = np.ascontiguousarray(
        whT.reshape(KT, P, H).transpose(1, 0, 2).reshape(P, KT * H))
    qT = np.asarray(query, np.float16).transpose(0, 2, 1)  # [B, H, T]
    # maskT[p, (b, u)] = 1.0 if u*128+p < len[b] else 0.0
    maskT = np.zeros((P, B, ST), np.float32)
    for b in range(B):
        m01 = (np.arange(S) < int(src_lengths[b])).astype(np.float32)
        maskT[:, b, :] = m01.reshape(ST, P).T
    maskT = np.ascontiguousarray(maskT.reshape(P, B * ST))
    in_maps = []
    for c in range(NCORES):
        # queryTs[p, (b, k, t)] = query[b, c*16+t, k*128+p]
        qc = qT[:, :, c * TLOC:(c + 1) * TLOC]
        qc = np.ascontiguousarray(
            qc.reshape(B, KT, P, TLOC).transpose(2, 0, 1, 3).reshape(P, B * KT * TLOC))
        in_maps.append({
            "queryTs": qc,
            "encTs": encTs, "encs": encs,
            "WsT": wsT, "WhT": whT, "vcol": vcol, "maskT": maskT,
        })
    return in_maps


def combine_v5(results, plan, ncols):
    out = np.zeros((B, T, H), np.float32)
    outw = KT * TLOC + ST * TLOC
    for c in range(NCORES):
        blob = results[c]["outb"].reshape(P, B, outw)
        for pb in plan:
            b = pb["b"]
            nh = len(pb["halves"])
            ctxT = blob[:, b, 0:KT * TLOC].reshape(P, KT, TLOC)  # [p, hb, t]
            den = blob[0, b, KT * TLOC:KT * TLOC + TLOC * nh]
            d = den.reshape(nh, TLOC).sum(axis=0)
            cpart = ctxT.transpose(1, 0, 2).reshape(H, TLOC)     # [h, t]
            out[b, c * TLOC:(c + 1) * TLOC, :] = (cpart / d[None, :]).T
    return out


_NC_V6 = {}


def _kernel_v6(query, encoder_outputs, src_lengths, Ws, Wh, v):
    key = tuple(int(x) for x in np.asarray(src_lengths))
    if key not in _NC_V6:
        _NC_V6[key] = build_nc_v5(key)
    nc, plan, ncols = _NC_V6[key]
    in_maps = make_in_maps_v5(query, encoder_outputs, src_lengths, Ws, Wh, v)
    res = run_bass_kernel_spmd(nc, in_maps, core_ids=list(range(NCORES)))
    return combine_v5(res.results, plan, ncols).astype(np.float32)


def kernel(query, encoder_outputs, src_lengths, Ws, Wh, v):
    try:
        return _kernel_v6(query, encoder_outputs, src_lengths, Ws, Wh, v)
    except Exception:
        return _kernel_v4(query, encoder_outputs, src_lengths, Ws, Wh, v)


# ===================== v4: length-aware T-sharded build =====================
# Every core runs the SAME program over ALL B batches but only TLOC=T/8 of
# the t rows; per-batch s-extents (from src_lengths) are baked in as static
# code, so dead source positions cost nothing. Masking falls out of exact
# stationary widths plus a -40 PSUM memset (exp -> 0). The context is
# produced transposed (ctxT[h, (hb, b, t)]) and UNNORMALIZED together with
# the per-(b,u,t) denominator row; the host does the final divide and
# re-layout. Compiled lazily per src_lengths tuple.

TLOC = T // NCORES   # 16 t rows per core
TBV = 8              # t-block for ACT batching (2 blocks per batch)


def _plan(lengths):
    lengths = [int(x) for x in lengths]
    plan = []
    col = 0
    for b, ln in enumerate(lengths):
        ln_c = (ln + 1) // 2 * 2          # pad compute extent to even (f16 align)
        halves = []                        # (u, m_bu, col_offset)
        for u in range(ST):
            m = min(P, ln - u * P)
            if m > 0:
                halves.append((u, m, col))
                col += TLOC
        plan.append({"b": b, "len": ln, "len_c": min(ln_c, S), "halves": halves})
    return plan, col                      # col = total scT columns (16 * sum halves)


def _build_kernel_v4(tc, ctx, aps, plan, ncols):
    nc = tc.nc
    f32 = dt.float32
    f16 = dt.float16

    const = ctx.enter_context(tc.tile_pool(name="const", bufs=1))
    psP = ctx.enter_context(tc.tile_pool(name="psP", bufs=2, space="PSUM"))
    psS = ctx.enter_context(tc.tile_pool(name="psS", bufs=1, space="PSUM"))

    # ---- inputs: all host-repacked partition-major, contiguous rows ----
    wsT_sb = const.tile([P, KT, H], f16)
    whT_sb = const.tile([P, KT, H], f16)
    vcol_sb = const.tile([P, KT], f16)
    nc.sync.dma_start(vcol_sb[:], aps["vcol"][:, :])
    qTs_sb = const.tile([P, B, KT, TLOC], f16)
    enc_all = const.tile([P, B, ST, H], f16)
    encT_all = const.tile([P, B, KT, S], f16)
    qTs_r = aps["queryTs"].rearrange("b p x -> b p x")
    encT_r = aps["encTs"].rearrange("b p x -> b p x")
    encs_r = aps["encs"].rearrange("b p x -> b p x")
    for b in range(B):
        nc.sync.dma_start(
            qTs_sb[:, b].rearrange("p k t -> p (k t)"), qTs_r[b])
        nc.sync.dma_start(
            encT_all[:, b].rearrange("p k s -> p (k s)"), encT_r[b])
        nc.sync.dma_start(
            enc_all[:, b].rearrange("p u h -> p (u h)"), encs_r[b])
        if b == 0:
            nc.sync.dma_start(
                wsT_sb[:].rearrange("p k o -> p (k o)"), aps["WsT"][:, :])
            nc.sync.dma_start(
                whT_sb[:].rearrange("p k o -> p (k o)"), aps["WhT"][:, :])
    encT_sb = [encT_all[:, b] for b in range(B)]
    u_index = {}
    for pb in plan:
        for i, (u, m, _) in enumerate(pb["halves"]):
            u_index[(pb["b"], i)] = u

    # ---- scores: scT[s, col(b,u,t)] in one PSUM tile --------------------
    scT_ps = psS.tile([P, ncols], f32, name="scT")
    nc.vector.memset(scT_ps[:], -40.0)
    tanh_pool = ctx.enter_context(tc.tile_pool(name="tanh", bufs=4))

    # Projections are software-pipelined ONE BATCH AHEAD of the score
    # loop: PE's in-order stream would otherwise place proj(b+1) after
    # scores(b) (which wait on tanh(b)), stalling the next batch's adds
    # and opening ACT gaps at every batch boundary.
    q_projT = const.tile([P, B, KT, TLOC], f32)
    e_projT = []

    def emit_proj(pb):
        b, ln_c = pb["b"], pb["len_c"]
        for j in range(KT):
            qp_ps = psP.tile([P, TLOC], f32, tag="qp", name=f"qp{b}_{j}")
            for k in range(KT):
                nc.tensor.matmul(
                    qp_ps[:], lhsT=wsT_sb[:, k, j * P:(j + 1) * P],
                    rhs=qTs_sb[:, b, k, :], start=(k == 0), stop=(k == KT - 1))
            nc.scalar.copy(q_projT[:, b, j, :], qp_ps[:])
        ep = const.tile([P, KT, ln_c], f16, name=f"eprojT{b}", tag=f"eprojT{b}")
        for j in range(KT):
            ep_ps = psP.tile([P, S], f32, tag="ep", name=f"ep{b}_{j}")
            for k in range(KT):
                nc.tensor.matmul(
                    ep_ps[:, 0:ln_c], lhsT=whT_sb[:, k, j * P:(j + 1) * P],
                    rhs=encT_sb[b][:, k, 0:ln_c], start=(k == 0), stop=(k == KT - 1))
            nc.scalar.copy(ep[:, j, :], ep_ps[:, 0:ln_c])
        e_projT.append(ep)

    emit_proj(plan[0])
    for bi, pb in enumerate(plan):
        b, ln_c = pb["b"], pb["len_c"]
        if bi + 1 < len(plan):
            emit_proj(plan[bi + 1])

        for tb in range(TLOC // TBV):
            t0 = tb * TBV
            tin = tanh_pool.tile([P, TBV, KT, ln_c], f16, tag="tin", name=f"tin{b}_{tb}")
            tout = tanh_pool.tile([P, TBV, KT, ln_c], f16, tag="tout", name=f"tout{b}_{tb}")
            for tl in range(TBV):
                for j in range(KT):
                    nc.vector.tensor_scalar_add(
                        tin[:, tl, j, :], e_projT[b][:, j, :],
                        q_projT[:, b, j, t0 + tl:t0 + tl + 1])
            nc.scalar.activation(tout[:], tin[:], AF.Tanh)
            for tl in range(TBV):
                for (u, m, col) in pb["halves"]:
                    cc = col + t0 + tl
                    for j in range(KT):
                        nc.tensor.matmul(
                            scT_ps[0:m, cc:cc + 1],
                            lhsT=tout[:, tl, j, u * P:u * P + m],
                            rhs=vcol_sb[:, j:j + 1],
                            start=(j == 0), stop=(j == KT - 1))

    # ---- exp + denominator ---------------------------------------------
    expT_sb = const.tile([P, ncols], f16)
    nc.scalar.activation(expT_sb[:], scT_ps[:], AF.Exp)
    ones_sb = const.tile([P, 1], f16)
    nc.vector.memset(ones_sb[:], 1.0)
    den_ps = psP.tile([1, ncols], f32, tag="den")
    nc.tensor.matmul(den_ps[:], lhsT=ones_sb[:], rhs=expT_sb[:])
    den_sb = const.tile([1, ncols], f32)
    nc.vector.tensor_copy(den_sb[:], den_ps[:])
    nc.sync.dma_start(aps["den"][:, :], den_sb[:])

    # ---- context (transposed, unnormalized) -----------------------------
    # ctxT_ps[p, (hb, b, t)] = sum_s enc[b][s, hb*128+p] * expT[s, col(b,u,t)]
    ctxT_ps = psS.tile([P, KT * B * TLOC], f32, name="ctxT")
    for pb in plan:
        b = pb["b"]
        nh = len(pb["halves"])
        for hb in range(KT):
            for i, (u, m, col) in enumerate(pb["halves"]):
                nc.tensor.matmul(
                    ctxT_ps[:, (hb * B + b) * TLOC:(hb * B + b + 1) * TLOC],
                    lhsT=enc_all[:, b, u, hb * P:(hb + 1) * P],
                    rhs=expT_sb[:, col:col + TLOC],
                    start=(i == 0), stop=(i == nh - 1))
    ctxT_sb = const.tile([P, KT * B * TLOC], f32)
    nc.vector.tensor_copy(ctxT_sb[:], ctxT_ps[:])
    nc.sync.dma_start(aps["ctxT"][:, :], ctxT_sb[:])


def build_nc_v4(lengths):
    plan, ncols = _plan(lengths)
    nc = bacc.Bacc("TRN2", target_bir_lowering=False, debug=False)
    aps = {
        "queryTs": nc.dram_tensor("queryTs", [B, P, KT * TLOC], dt.float16, kind="ExternalInput").ap(),
        "encTs": nc.dram_tensor("encTs", [B, P, KT * S], dt.float16, kind="ExternalInput").ap(),
        "encs": nc.dram_tensor("encs", [B, P, ST * H], dt.float16, kind="ExternalInput").ap(),
        "WsT": nc.dram_tensor("WsT", [P, KT * H], dt.float16, kind="ExternalInput").ap(),
        "WhT": nc.dram_tensor("WhT", [P, KT * H], dt.float16, kind="ExternalInput").ap(),
        "vcol": nc.dram_tensor("vcol", [P, KT], dt.float16, kind="ExternalInput").ap(),
        "den": nc.dram_tensor("den", [1, ncols], dt.float32, kind="ExternalOutput").ap(),
        "ctxT": nc.dram_tensor("ctxT", [P, KT * B * TLOC], dt.float32, kind="ExternalOutput").ap(),
    }
    with ExitStack() as ctx:
        with tile.TileContext(nc) as tc:
            _build_kernel_v4(tc, ctx, aps, plan, ncols)
            ctx.close()
    nc.compile()
    return nc, plan, ncols


def make_in_maps_v4(query, encoder_outputs, src_lengths, Ws, Wh, v):
    wsT = np.ascontiguousarray(Ws.T).astype(np.float16)
    whT = np.ascontiguousarray(Wh.T).astype(np.float16)
    vcol = np.ascontiguousarray(np.asarray(v, np.float32).reshape(KT, P).T).astype(np.float16)
    enc16 = np.asarray(encoder_outputs, np.float16)
    # encTs[b, p, (k, s)] = enc[b, s, k*128+p]
    encTs = np.ascontiguousarray(
        enc16.transpose(0, 2, 1).reshape(B, KT, P, S).transpose(0, 2, 1, 3)
        .reshape(B, P, KT * S))
    # encs[b, p, (u, h)] = enc[b, u*128+p, h]
    encs = np.ascontiguousarray(
        enc16.reshape(B, ST, P, H).transpose(0, 2, 1, 3).reshape(B, P, ST * H))
    # wsT2[p, (k, o)] = Ws.T[k*128+p, o]
    wsT = np.ascontiguousarray(
        wsT.reshape(KT, P, H).transpose(1, 0, 2).reshape(P, KT * H))
    whT = np.ascontiguousarray(
        whT.reshape(KT, P, H).transpose(1, 0, 2).reshape(P, KT * H))
    qT = np.asarray(query, np.float16).transpose(0, 2, 1)  # [B, H, T]
    in_maps = []
    for c in range(NCORES):
        # queryTs[b, p, (k, t)] = query[b, c*16+t, k*128+p]
        qc = qT[:, :, c * TLOC:(c + 1) * TLOC]
        qc = np.ascontiguousarray(
            qc.reshape(B, KT, P, TLOC).transpose(0, 2, 1, 3).reshape(B, P, KT * TLOC))
        in_maps.append({
            "queryTs": qc,
            "encTs": encTs, "encs": encs,
            "WsT": wsT, "WhT": whT, "vcol": vcol,
        })
    return in_maps


def combine_v4(results, plan, ncols):
    out = np.zeros((B, T, H), np.float32)
    for c in range(NCORES):
        ctxT = results[c]["ctxT"].reshape(P, KT, B, TLOC)   # [p, hb, b, t]
        den = results[c]["den"].reshape(ncols)
        for pb in plan:
            b = pb["b"]
            d = np.zeros(TLOC, np.float32)
            for (u, m, col) in pb["halves"]:
                d += den[col:col + TLOC]
            # ctx[t, h] with h = hb*128 + p
            cpart = ctxT[:, :, b, :].transpose(1, 0, 2).reshape(H, TLOC)
            out[b, c * TLOC:(c + 1) * TLOC, :] = (cpart / d[None, :]).T
    return out


_NC_V4 = {}


def _kernel_v4(query, encoder_outputs, src_lengths, Ws, Wh, v):
    key = tuple(int(x) for x in np.asarray(src_lengths))
    if key not in _NC_V4:
        _NC_V4[key] = build_nc_v4(key)
    nc, plan, ncols = _NC_V4[key]
    in_maps = make_in_maps_v4(query, encoder_outputs, src_lengths, Ws, Wh, v)
    res = run_bass_kernel_spmd(nc, in_maps, core_ids=list(range(NCORES)))
    return combine_v4(res.results, plan, ncols).astype(np.float32)


